# revision 19
# baseline (speedup 1.0000x reference)
"""Bidirectional-GRU encoding layer for Trainium2 (8 NeuronCores, Bass/Tile).

The reference computes a length-masked bidirectional GRU over [B=32, T=512,
D=512] and returns gru_outputs[:, -1, :] (shape [B, 2H]).  dynamic_rnn
masking means output rows are exactly zero for every sample with
length < T, and for samples with length == T the row is
    [ fw_h_after_T_steps , (1-u)*c of a single bw GRU step on x[T-1] ].
The kernel computes exactly that: a single-step bw-GRU candidate for all
samples (masked by length==T) always runs on-device; the 512-step fw scan
is only compiled/run when at least one sample has length == T.

Sharding: data-parallel over batch, 4 samples per core (weights replicated).
Compute layout is feature-on-partition (everything transposed), so the
sequential scan's elementwise chain runs on [128, few] tiles.  Matmul
operands (weights, x, h state) are fp16 with fp32 PSUM accumulation and
fp32 gate math — fp32 matmuls cost two PE passes on trn2 and the scan is
weight-load-bound; fp16 keeps the end-to-end error ~6e-4.  The u-gate
weight columns are pre-negated on the host so sigmoid yields v = 1-u
directly, shortening the post-tanh critical path of each scan step.
"""

import numpy as np

B, T, D, H = 32, 512, 512, 512
N_CORES = 8
BPC = B // N_CORES  # 4 samples per core
P = 128
KD = D // P  # 4 k-tiles over the depth dim
MH = H // P  # 4 m-tiles over the hidden dim
NG = (2 * H + H) // P  # 12 m-tiles over [ru | c] gate outputs

SCAN_UNROLL = 16
_CACHE = {}
TRACE = False          # test harness sets True to capture an NTFF profile
LAST_RESULT = None     # BassKernelResults of the most recent run


def _bf16():
    return np.float16


def _build_kernel(with_scan: bool):
    import concourse.mybir as mybir
    import concourse.tile as tile
    from concourse import bacc
    from concourse.bass import ds, ts

    f32 = mybir.dt.float32
    bf16 = mybir.dt.float16
    AF = mybir.ActivationFunctionType

    nc = bacc.Bacc("TRN2", target_bir_lowering=False, debug=False,
                   num_devices=N_CORES)

    # --- DRAM I/O (per-core shards; weights replicated) ---
    xlastT_d = nc.dram_tensor("xlastT", [D, BPC], bf16, kind="ExternalInput").ap()
    maskg_d = nc.dram_tensor("maskg", [P, MH, BPC], f32, kind="ExternalInput").ap()
    bwWu_d = nc.dram_tensor("bwWu", [D, H], bf16, kind="ExternalInput").ap()
    bwWc_d = nc.dram_tensor("bwWc", [D, H], bf16, kind="ExternalInput").ap()
    bwbu_d = nc.dram_tensor("bwbu", [P, MH], f32, kind="ExternalInput").ap()
    bwbc_d = nc.dram_tensor("bwbc", [P, MH], f32, kind="ExternalInput").ap()
    if with_scan:
        fwWx_d = nc.dram_tensor("fwWx", [D, 3 * H], bf16, kind="ExternalInput").ap()
        fwWh_d = nc.dram_tensor("fwWh", [H, 3 * H], bf16, kind="ExternalInput").ap()
        fwb_d = nc.dram_tensor("fwb", [P, NG], f32, kind="ExternalInput").ap()
        xscanT_d = nc.dram_tensor("xscanT", [BPC, D, T], bf16,
                                  kind="ExternalInput").ap()
    outT_d = nc.dram_tensor("outT", [2 * H, BPC], f32, kind="ExternalOutput").ap()
    # view as [P, 8, BPC]: row (a*128+p) -> [p, a, s]; a=0..3 fw, a=4..7 bw
    out_v = outT_d.rearrange("(a p) s -> p a s", p=P)

    with tile.TileContext(nc) as tc:
        with (
            tc.tile_pool(name="const", bufs=1) as cpool,
            tc.tile_pool(name="work", bufs=4) as wpool,
        ):
            # ---------- Phase A: single-step bw candidate, masked ----------
            xlast = cpool.tile([P, KD, BPC], bf16, tag="xlast")
            nc.sync.dma_start(xlast[:], xlastT_d.rearrange("(k p) s -> p k s", p=P))
            mask = cpool.tile([P, MH, BPC], f32, tag="mask")
            nc.sync.dma_start(mask[:], maskg_d[:])
            bwWu = cpool.tile([P, KD, H], bf16, tag="bwWu")
            nc.sync.dma_start(bwWu[:], bwWu_d.rearrange("(k p) m -> p k m", p=P))
            bwWc = cpool.tile([P, KD, H], bf16, tag="bwWc")
            nc.sync.dma_start(bwWc[:], bwWc_d.rearrange("(k p) m -> p k m", p=P))
            bwbu = cpool.tile([P, MH], f32, tag="bwbu")
            nc.sync.dma_start(bwbu[:], bwbu_d[:])
            bwbc = cpool.tile([P, MH], f32, tag="bwbc")
            nc.sync.dma_start(bwbc[:], bwbc_d[:])

            with tc.tile_pool(name="psumA", bufs=1, space="PSUM") as ppoolA:
                pu = ppoolA.tile([P, MH, BPC], f32, tag="pu")
                pc = ppoolA.tile([P, MH, BPC], f32, tag="pc")
                for m in range(MH):
                    for k in range(KD):
                        nc.tensor.matmul(pu[:, m, :], bwWu[:, k, ts(m, P)],
                                         xlast[:, k, :], start=(k == 0),
                                         stop=(k == KD - 1))
                    for k in range(KD):
                        nc.tensor.matmul(pc[:, m, :], bwWc[:, k, ts(m, P)],
                                         xlast[:, k, :], start=(k == 0),
                                         stop=(k == KD - 1))
                u1 = wpool.tile([P, MH, BPC], f32, tag="u1")   # 1-u = sigmoid(-z)
                cc = wpool.tile([P, MH, BPC], f32, tag="cc")
                for m in range(MH):
                    nc.scalar.activation(u1[:, m, :], pu[:, m, :], AF.Sigmoid,
                                         bias=bwbu[:, m:m + 1])
                    nc.scalar.activation(cc[:, m, :], pc[:, m, :], AF.Tanh,
                                         bias=bwbc[:, m:m + 1])
                bwcand = wpool.tile([P, MH, BPC], f32, tag="bwcand")
                nc.vector.tensor_mul(bwcand[:], u1[:], cc[:])
                bwout = wpool.tile([P, MH, BPC], f32, tag="bwout")
                nc.vector.tensor_mul(bwout[:], bwcand[:], mask[:])
                nc.sync.dma_start(out_v[:, MH:2 * MH, :], bwout[:])

            if not with_scan:
                # fw half of the output is exactly zero (no length==T sample)
                zf = wpool.tile([P, MH, BPC], f32, tag="zfw")
                nc.vector.memset(zf[:], 0.0)
                nc.sync.dma_start(out_v[:, 0:MH, :], zf[:])

            # ---------- Phase B: x-projections for all t (if scanning) -----
            if with_scan:
                fwb = cpool.tile([P, NG], f32, tag="fwb")
                nc.sync.dma_start(fwb[:], fwb_d[:])
                fwWh = cpool.tile([P, KD, 3 * H], bf16, tag="fwWh")
                nc.sync.dma_start(fwWh[:], fwWh_d.rearrange("(k p) m -> p k m", p=P))

                # XG[p, t, m, s] = (x_s[t] @ fwWx + fwb)[m*128+p]
                XG = cpool.tile([P, T, NG, BPC], f32, tag="XG")
                with (
                    tc.tile_pool(name="xpre", bufs=2) as xpool,
                    tc.tile_pool(name="psumB", bufs=4, space="PSUM") as ppoolB,
                ):
                    fwWx = xpool.tile([P, KD, 3 * H], bf16, tag="fwWx")
                    nc.sync.dma_start(fwWx[:],
                                      fwWx_d.rearrange("(k p) m -> p k m", p=P))
                    for s in range(BPC):
                        xs = xpool.tile([P, KD, T], bf16, tag="xs")
                        nc.sync.dma_start(
                            xs[:], xscanT_d[s].rearrange("(k p) t -> p k t", p=P))
                        for m in range(NG):
                            pxg = ppoolB.tile([P, T], f32, tag="pxg")
                            for k in range(KD):
                                nc.tensor.matmul(pxg[:], fwWx[:, k, ts(m, P)],
                                                 xs[:, k, :], start=(k == 0),
                                                 stop=(k == KD - 1))
                            nc.scalar.activation(XG[:, :, m, s], pxg[:],
                                                 AF.Identity, bias=fwb[:, m:m + 1])

                # ---------- Phase C: the sequential scan -------------------
                # state lives in fp16 (matmul operand dtype) the whole time
                hT = cpool.tile([P, 1, MH, BPC], bf16, tag="hT")
                nc.vector.memset(hT[:], 0.0)

                with tc.tile_pool(name="psumC", bufs=2, space="PSUM") as ppoolC:

                    def step(t):
                        # r gates first: the c-matmuls depend only on r*h
                        pg_r = ppoolC.tile([P, 1, MH, BPC], f32, tag="pg_r")
                        for m in range(MH):
                            for k in range(KD):
                                nc.tensor.matmul(pg_r[:, 0, m, :],
                                                 fwWh[:, k, ts(m, P)],
                                                 hT[:, 0, k, :], start=(k == 0),
                                                 stop=(k == KD - 1))
                        zg_r = wpool.tile([P, 1, MH, BPC], f32, tag="zg_r")
                        nc.vector.tensor_add(zg_r[:], pg_r[:],
                                             XG[:, ds(t, 1), 0:MH, :])
                        g_r = wpool.tile([P, 1, MH, BPC], f32, tag="g_r")
                        nc.scalar.activation(g_r[:], zg_r[:], AF.Sigmoid)
                        rh = wpool.tile([P, 1, MH, BPC], bf16, tag="rh")
                        nc.vector.tensor_mul(rh[:], g_r[:], hT[:])

                        # v = 1-u gates (u-columns of Whg/XG pre-negated on
                        # host); on the PE these run while the r-gate chain
                        # (zg_r -> sigmoid -> rh) is in flight
                        pg_v = ppoolC.tile([P, 1, MH, BPC], f32, tag="pg_v")
                        for m in range(MH):
                            for k in range(KD):
                                nc.tensor.matmul(pg_v[:, 0, m, :],
                                                 fwWh[:, k, ts(MH + m, P)],
                                                 hT[:, 0, k, :], start=(k == 0),
                                                 stop=(k == KD - 1))
                        zg_v = wpool.tile([P, 1, MH, BPC], f32, tag="zg_v")
                        nc.vector.tensor_add(zg_v[:], pg_v[:],
                                             XG[:, ds(t, 1), MH:2 * MH, :])
                        g_v = wpool.tile([P, 1, MH, BPC], f32, tag="g_v")
                        nc.scalar.activation(g_v[:], zg_v[:], AF.Sigmoid)

                        pcs = ppoolC.tile([P, 1, MH, BPC], f32, tag="pcs")
                        for m in range(MH):
                            for k in range(KD):
                                nc.tensor.matmul(pcs[:, 0, m, :],
                                                 fwWh[:, k, ds(2 * H + m * P, P)],
                                                 rh[:, 0, k, :], start=(k == 0),
                                                 stop=(k == KD - 1))
                        zc = wpool.tile([P, 1, MH, BPC], f32, tag="zc")
                        nc.vector.tensor_add(zc[:], pcs[:],
                                             XG[:, ds(t, 1), 2 * MH:NG, :])
                        # a = u*h = h - v*h, off the critical path (overlaps
                        # the c-matmuls / tanh)
                        a2 = wpool.tile([P, 1, MH, BPC], f32, tag="a2")
                        nc.vector.tensor_mul(a2[:], g_v[:], hT[:])
                        ah = wpool.tile([P, 1, MH, BPC], f32, tag="ah")
                        nc.vector.tensor_sub(ah[:], hT[:], a2[:])
                        ct = wpool.tile([P, 1, MH, BPC], f32, tag="ct")
                        nc.scalar.activation(ct[:], zc[:], AF.Tanh)
                        bt = wpool.tile([P, 1, MH, BPC], f32, tag="bt")
                        nc.vector.tensor_mul(bt[:], g_v[:], ct[:])
                        # h' = u*h + (1-u)*c, rounded to fp16 state
                        nc.vector.tensor_add(hT[:], ah[:], bt[:])

                    if SCAN_UNROLL > 1:
                        def ubody(iv0, unroll):
                            for i in range(unroll):
                                step(iv0 + i)
                        tc.For_i_unrolled_general(
                            0, T, 1, ubody, max_unroll=SCAN_UNROLL,
                            hint_engines=(mybir.EngineType.PE,
                                          mybir.EngineType.DVE))
                    else:
                        with tc.For_i(0, T, 1) as t:
                            step(t)

                fwout = wpool.tile([P, MH, BPC], f32, tag="fwout")
                nc.vector.tensor_mul(fwout[:], hT[:, 0, :, :], mask[:])
                nc.sync.dma_start(out_v[:, 0:MH, :], fwout[:])

    nc.compile()
    return nc


def _get_kernel(with_scan: bool):
    key = ("scan" if with_scan else "noscan")
    if key not in _CACHE:
        _CACHE[key] = _build_kernel(with_scan)
    return _CACHE[key]


def host_inputs(inputs, fw_gk, fw_gb, fw_ck, fw_cb,
                bw_gk, bw_gb, bw_ck, bw_cb, length):
    """Shard/transpose/cast the full inputs into per-core in_maps."""
    bf16 = _bf16()
    inputs = np.asarray(inputs, dtype=np.float32)
    length = np.asarray(length)
    mask = (length.astype(np.int64) >= T).astype(np.float32)  # [B]
    with_scan = bool(mask.any())

    fw_gk = np.asarray(fw_gk, np.float32)
    fw_ck = np.asarray(fw_ck, np.float32)
    bw_gk = np.asarray(bw_gk, np.float32)
    bw_ck = np.asarray(bw_ck, np.float32)
    fw_gb = np.asarray(fw_gb, np.float32)
    fw_cb = np.asarray(fw_cb, np.float32)
    bw_gb = np.asarray(bw_gb, np.float32)
    bw_cb = np.asarray(bw_cb, np.float32)

    # Replicated weight shards.
    shared = {
        "bwWu": np.ascontiguousarray((-bw_gk[:D, H:2 * H]).astype(bf16)),
        "bwWc": np.ascontiguousarray(bw_ck[:D].astype(bf16)),
        "bwbu": np.ascontiguousarray((-bw_gb[H:2 * H]).reshape(MH, P).T),
        "bwbc": np.ascontiguousarray(bw_cb.reshape(MH, P).T),
    }
    if with_scan:
        # u-gate columns pre-negated: sigmoid then yields v = 1-u directly
        neg = np.ones((1, 3 * H), np.float32)
        neg[:, H:2 * H] = -1.0
        shared["fwWx"] = np.ascontiguousarray(
            (np.concatenate([fw_gk[:D], fw_ck[:D]], axis=1) * neg).astype(bf16))
        shared["fwWh"] = np.ascontiguousarray(
            (np.concatenate([fw_gk[D:], fw_ck[D:]], axis=1) * neg).astype(bf16))
        fwb_full = np.concatenate([fw_gb, fw_cb]) * neg[0]
        shared["fwb"] = np.ascontiguousarray(fwb_full.reshape(NG, P).T)

    in_maps = []
    for c in range(N_CORES):
        sl = slice(c * BPC, (c + 1) * BPC)
        m = dict(shared)
        m["xlastT"] = np.ascontiguousarray(inputs[sl, T - 1, :].T.astype(bf16))
        m["maskg"] = np.ascontiguousarray(
            np.broadcast_to(mask[sl][None, None, :], (P, MH, BPC)))
        if with_scan:
            m["xscanT"] = np.ascontiguousarray(
                inputs[sl].transpose(0, 2, 1).astype(bf16))
        in_maps.append(m)
    return with_scan, in_maps


def kernel(inputs, fw_gk, fw_gb, fw_ck, fw_cb,
           bw_gk, bw_gb, bw_ck, bw_cb, length):
    from concourse.bass_utils import run_bass_kernel_spmd

    with_scan, in_maps = host_inputs(inputs, fw_gk, fw_gb, fw_ck, fw_cb,
                                     bw_gk, bw_gb, bw_ck, bw_cb, length)
    nc = _get_kernel(with_scan)
    res = run_bass_kernel_spmd(nc, in_maps, core_ids=list(range(N_CORES)),
                               trace=TRACE)
    global LAST_RESULT
    LAST_RESULT = res

    out = np.empty((B, 2 * H), np.float32)
    for c in range(N_CORES):
        out[c * BPC:(c + 1) * BPC] = res.results[c]["outT"].T
    return out
